# revision 12
# baseline (speedup 1.0000x reference)
"""Trainium2 Bass kernel for causal GQA self-attention (fused QKV + RoPE).

Problem: B=2, T=2048, C=2048, H=16 q-heads, KV=4 kv-heads, HD=128.
Sharding: 8 cores = (batch b, kv-group k). Each core computes the 4 q-heads
of one kv group for one batch element; outputs are disjoint slices of y.

Per-core device kernel (all fp32r matmuls, ~14-bit mantissa rounding):
  1. QKV projection qkv^T = W_shard @ x^T, d-major layout [j, t].
     Host pre-transposes x and W (and pre-permutes q/k head dims so RoPE
     becomes rotate-half instead of interleaved pairs).
  2. RoPE on q/k via SBUF->SBUF DMA partition swap + DVE mul/add.
  3. Attention in S^T orientation: scores^T[s,t] = K^T.T @ Q^T per
     (128 s-chunk x 512 t-block), exp on ScalarE, causal diagonal masked by a
     triangular multiply, row sums via an all-ones stationary matmul
     (partition reduction on the PE), PV with V stationary accumulating
     y^T[d,t] in PSUM, then normalize by reciprocal row sums.
Output per core: y^T [512, 2048]; host transposes and concatenates.
"""

import math

import numpy as np

import concourse.bass as bass
import concourse.mybir as mybir
import concourse.tile as tile
from concourse import bacc
from concourse.bass_utils import run_bass_kernel_spmd

B, T, C = 2, 2048, 2048
H, KV, HD = 16, 4, 128
NREP = H // KV  # q heads per core
P = 128
NCORES = 8
CC_CHUNKS = C // P  # 16 contraction chunks
TT = 4  # t-blocks of 512
TB = T // TT  # 512
NB = 6  # j-blocks per core: q0..q3, k, v
SCALE = 1.0 / math.sqrt(HD)

f32 = mybir.dt.float32
f32r = mybir.dt.float32r

TRACE = False  # set True (with ntff shim installed) to get exec_time_ns

_cache = {}


def _build():
    if "nc" in _cache:
        return _cache["nc"]

    nc = bacc.Bacc("TRN2", target_bir_lowering=False, debug=False,
                   num_devices=NCORES)

    xT_d = nc.dram_tensor("xT", [P, CC_CHUNKS, T], f32r, kind="ExternalInput").ap()
    wT_d = nc.dram_tensor("wT", [P, CC_CHUNKS, NB * P], f32r, kind="ExternalInput").ap()
    cc_d = nc.dram_tensor("CC", [P, T], f32r, kind="ExternalInput").ap()
    ss_d = nc.dram_tensor("SS2", [P, T], f32r, kind="ExternalInput").ap()
    tri_d = nc.dram_tensor("tri", [P, P], f32r, kind="ExternalInput").ap()
    ones_d = nc.dram_tensor("ones", [P, P], f32r, kind="ExternalInput").ap()
    ident_d = nc.dram_tensor("ident", [P, P], f32r, kind="ExternalInput").ap()
    yT_d = nc.dram_tensor("yT", [NREP * P, T], f32, kind="ExternalOutput").ap()
    sums_d = nc.dram_tensor("sums", [NREP * TT, TB], f32, kind="ExternalOutput").ap()

    with tile.TileContext(nc) as tc:
        with (
            tc.tile_pool(name="wt", bufs=1) as wt_pool,
            tc.tile_pool(name="xt", bufs=4) as xt_pool,
            tc.tile_pool(name="qkvt", bufs=1) as qkv_pool,
            tc.tile_pool(name="freq", bufs=1) as freq_pool,
            tc.tile_pool(name="small", bufs=1) as small_pool,
            tc.tile_pool(name="vsb", bufs=1) as v_pool,
            tc.tile_pool(name="swp", bufs=2) as swp_pool,
            tc.tile_pool(name="ropetmp", bufs=2) as rt_pool,
            tc.tile_pool(name="expt", bufs=6) as exp_pool,
            tc.tile_pool(name="yout", bufs=2) as y_pool,
            tc.tile_pool(name="psum", bufs=8, space="PSUM") as psum_pool,
        ):
            # ---- resident tensors ----
            # interleave first x chunks with weight quarters so the first
            # matmuls start ~7us in and stalls stay under the HAM window
            xts_tt0 = [
                xt_pool.tile([P, 4, TB], f32r, tag="xt", name=f"xt0_{cq}")
                for cq in range(4)
            ]
            wt_q = [
                wt_pool.tile([P, 4, NB * P], f32r, tag=f"wt{wq}", name=f"wt{wq}")
                for wq in range(4)
            ]
            nc.sync.dma_start(xts_tt0[0][:], xT_d[:, 0:4, 0:TB])
            nc.sync.dma_start(wt_q[0][:], wT_d[:, 0:4, :])
            nc.sync.dma_start(wt_q[1][:], wT_d[:, 4:8, :])
            nc.sync.dma_start(xts_tt0[1][:], xT_d[:, 4:8, 0:TB])
            nc.sync.dma_start(wt_q[2][:], wT_d[:, 8:12, :])
            nc.sync.dma_start(xts_tt0[2][:], xT_d[:, 8:12, 0:TB])
            nc.sync.dma_start(wt_q[3][:], wT_d[:, 12:16, :])
            nc.sync.dma_start(xts_tt0[3][:], xT_d[:, 12:16, 0:TB])

            # qkv^T blocks [128 d, 2048 t]: jb 0..3 = q heads (rope-permuted),
            # 4 = k (rope-permuted), 5 = v
            qkvT = [
                qkv_pool.tile([P, T], f32r, tag=f"qkv{jb}", name=f"qkv{jb}")
                for jb in range(NB)
            ]
            # V in s-major: [128 s, 16 s-chunk, 128 d]
            v_sb = v_pool.tile([P, CC_CHUNKS, P], f32r, tag="vsb")

            # late-loaded constants (needed only after proj tt0)
            ccs = freq_pool.tile([P, T], f32r, tag="cc")
            ss2 = freq_pool.tile([P, T], f32r, tag="ss")
            tri = small_pool.tile([P, P], f32r, tag="tri")
            ones = small_pool.tile([P, P], f32r, tag="ones")
            ident = small_pool.tile([P, P], f32r, tag="ident")
            _late = [(ccs, cc_d), (ss2, ss_d), (tri, tri_d), (ones, ones_d),
                     (ident, ident_d)]

            # ---- projection: qkv^T accumulated over 16 c-chunks ----
            for tt in range(TT):
                proj_psums = [
                    psum_pool.tile([P, TB], f32, tag="mm", name="proj_ps")
                    for _ in range(NB)
                ]
                for cq in range(4):  # c-chunk quarters, N=512 matmuls
                    if tt == 0:
                        xt = xts_tt0[cq]
                    else:
                        xt = xt_pool.tile([P, 4, TB], f32r, tag="xt", name="xt")
                        nc.sync.dma_start(
                            xt[:], xT_d[:, cq * 4:(cq + 1) * 4, tt * TB:(tt + 1) * TB]
                        )
                    for ci in range(4):
                        cc = cq * 4 + ci
                        for jb in range(NB):
                            nc.tensor.matmul(
                                proj_psums[jb][:],
                                wt_q[cq][:, ci, jb * P:(jb + 1) * P],
                                xt[:, ci, :],
                                start=(cc == 0),
                                stop=(cc == CC_CHUNKS - 1),
                            )
                if tt == 0:
                    for _tile, _src in _late:
                        nc.sync.dma_start(_tile[:], _src[:])
                    _late = []
                if True:
                    tsl = slice(tt * TB, (tt + 1) * TB)
                    for jb in range(NB):
                        nc.vector.tensor_copy(qkvT[jb][:, tsl], proj_psums[jb][:])
                    # V transpose for this chunk: v^T [d, s] -> v_sb [s, d]
                    for i in range(4):
                        sc = 4 * tt + i
                        trp = psum_pool.tile([P, TB], f32r, tag="mm", name="trp")
                        nc.tensor.transpose(
                            trp[:, :P], qkvT[5][:, sc * P:(sc + 1) * P], ident[:]
                        )
                        nc.vector.tensor_copy(v_sb[:, sc, :], trp[:, :P])
                    # RoPE for q0..q3 and k on this t-chunk
                    for jb in range(5):
                        swp = swp_pool.tile([P, TB], f32r, tag="swp", name="swp")
                        nc.sync.dma_start(swp[0:64, :], qkvT[jb][64:128, tsl])
                        nc.sync.dma_start(swp[64:128, :], qkvT[jb][0:64, tsl])
                        ta = rt_pool.tile([P, TB], f32r, tag="ta", name="ta")
                        tb_ = rt_pool.tile([P, TB], f32r, tag="tb", name="tb")
                        nc.vector.tensor_tensor(
                            ta[:], qkvT[jb][:, tsl], ccs[:, tsl], mybir.AluOpType.mult
                        )
                        nc.vector.tensor_tensor(
                            tb_[:], swp[:], ss2[:, tsl], mybir.AluOpType.mult
                        )
                        nc.vector.tensor_tensor(
                            qkvT[jb][:, tsl], ta[:], tb_[:], mybir.AluOpType.add
                        )

            # ---- attention, S^T orientation ----
            for tb in range(TT):
                for h in range(NREP):
                    psum_y = psum_pool.tile([P, TB], f32, tag="mm", name="psum_y")
                    psum_sum = psum_pool.tile([P, TB], f32, tag="mm", name="psum_sum")
                    nsc = 4 * (tb + 1)
                    for sc in range(nsc):
                        r = sc - 4 * tb  # >=0: diagonal-crossing chunk
                        col0 = r * P if r >= 0 else 0
                        psum_s = psum_pool.tile([P, TB], f32, tag="mm", name="psum_s")
                        nc.tensor.matmul(
                            psum_s[:, col0:],
                            qkvT[4][:, sc * P:(sc + 1) * P],
                            qkvT[h][:, tb * TB + col0:(tb + 1) * TB],
                            start=True,
                            stop=True,
                        )
                        expt = exp_pool.tile([P, TB], f32r, tag="expt", name="expt")
                        nc.scalar.activation(
                            expt[:, col0:],
                            psum_s[:, col0:],
                            mybir.ActivationFunctionType.Exp,
                            scale=SCALE,
                        )
                        if r >= 0:
                            nc.vector.tensor_tensor(
                                expt[:, col0:col0 + P],
                                expt[:, col0:col0 + P],
                                tri[:],
                                mybir.AluOpType.mult,
                            )
                        nc.tensor.matmul(
                            psum_sum[:, col0:],
                            ones[:],
                            expt[:, col0:],
                            start=(sc == 0),
                            stop=(sc == nsc - 1),
                        )
                        nc.tensor.matmul(
                            psum_y[:, col0:],
                            v_sb[:, sc, :],
                            expt[:, col0:],
                            start=(sc == 0),
                            stop=(sc == nsc - 1),
                        )
                    y_sb = y_pool.tile([P, TB], f32, tag="ysb", name="ysb")
                    nc.vector.tensor_copy(y_sb[:], psum_y[:])
                    nc.sync.dma_start(
                        yT_d[h * P:(h + 1) * P, tb * TB:(tb + 1) * TB], y_sb[:]
                    )
                    sums_sb = y_pool.tile([1, TB], f32, tag="sums", name="sums_sb")
                    nc.vector.tensor_copy(sums_sb[:], psum_sum[0:1, :])
                    nc.sync.dma_start(
                        sums_d[h * TT + tb:h * TT + tb + 1, :], sums_sb[0:1, :]
                    )

    nc.compile()
    _cache["nc"] = nc
    return nc


def _host_prep(x, w_qkv, freqs_cos, freqs_sin):
    """Build per-core input maps (numpy, cheap)."""
    x = np.asarray(x, dtype=np.float32)
    w_qkv = np.asarray(w_qkv, dtype=np.float32)
    freqs_cos = np.asarray(freqs_cos, dtype=np.float32)
    freqs_sin = np.asarray(freqs_sin, dtype=np.float32)

    perm = np.concatenate([np.arange(0, HD, 2), np.arange(1, HD, 2)])

    xTs = []
    for b in range(B):
        xt = np.ascontiguousarray(
            x[b].T.reshape(CC_CHUNKS, P, T).transpose(1, 0, 2)
        )
        xTs.append(xt)

    cosT = freqs_cos.T  # [64, T]
    sinT = freqs_sin.T
    CCh = np.ascontiguousarray(np.concatenate([cosT, cosT], axis=0))
    SS2 = np.ascontiguousarray(np.concatenate([-sinT, sinT], axis=0))
    tri = np.triu(np.ones((P, P), dtype=np.float32))
    ones = np.ones((P, P), dtype=np.float32)
    ident = np.eye(P, dtype=np.float32)

    in_maps = []
    for core in range(NCORES):
        b, kv = divmod(core, KV)
        blocks = []
        for r in range(NREP):
            hrow = (kv * NREP + r) * HD
            blocks.append(w_qkv[hrow:hrow + HD][perm])
        blocks.append(w_qkv[H * HD + kv * HD:H * HD + (kv + 1) * HD][perm])
        blocks.append(
            w_qkv[(H + KV) * HD + kv * HD:(H + KV) * HD + (kv + 1) * HD]
        )
        w_shard = np.concatenate(blocks, axis=0)  # [768, C]
        wT = np.ascontiguousarray(
            w_shard.T.reshape(CC_CHUNKS, P, NB * P).transpose(1, 0, 2)
        )
        in_maps.append({
            "xT": xTs[b],
            "wT": wT,
            "CC": CCh,
            "SS2": SS2,
            "tri": tri,
            "ones": ones,
            "ident": ident,
        })
    return in_maps


def kernel(x, w_qkv, freqs_cos, freqs_sin):
    nc = _build()
    in_maps = _host_prep(x, w_qkv, freqs_cos, freqs_sin)
    res = run_bass_kernel_spmd(nc, in_maps, list(range(NCORES)), trace=TRACE)
    _cache["last_res"] = res

    y = np.empty((B, T, C), dtype=np.float32)
    for core in range(NCORES):
        b, kv = divmod(core, KV)
        yT = res.results[core]["yT"]  # [NREP*P, T] unnormalized
        sums = res.results[core]["sums"].reshape(NREP, T)  # per (h, t)
        yT = yT.reshape(NREP, P, T) / sums[:, None, :]
        y[b, :, kv * NREP * HD:(kv + 1) * NREP * HD] = (
            yT.reshape(NREP * P, T).T
        )
    return y


# revision 13
# speedup vs baseline: 1.0290x; 1.0290x over previous
"""Trainium2 Bass kernel for causal GQA self-attention (fused QKV + RoPE).

Problem: B=2, T=2048, C=2048, H=16 q-heads, KV=4 kv-heads, HD=128.
Sharding: 8 cores = (batch b, kv-group k). Each core computes the 4 q-heads
of one kv group for one batch element; outputs are disjoint slices of y.

Per-core device kernel (all fp32r matmuls, ~14-bit mantissa rounding):
  1. QKV projection qkv^T = W_shard @ x^T, d-major layout [j, t].
     Host pre-transposes x and W (and pre-permutes q/k head dims so RoPE
     becomes rotate-half instead of interleaved pairs).
  2. RoPE on q/k via SBUF->SBUF DMA partition swap + DVE mul/add.
  3. Attention in S^T orientation: scores^T[s,t] = K^T.T @ Q^T per
     (128 s-chunk x 512 t-block), exp on ScalarE, causal diagonal masked by a
     triangular multiply, row sums via an all-ones stationary matmul
     (partition reduction on the PE), PV with V stationary accumulating
     y^T[d,t] in PSUM, then normalize by reciprocal row sums.
Output per core: y^T [512, 2048]; host transposes and concatenates.
"""

import math

import numpy as np

import concourse.bass as bass
import concourse.mybir as mybir
import concourse.tile as tile
from concourse import bacc
from concourse.bass_utils import run_bass_kernel_spmd

B, T, C = 2, 2048, 2048
H, KV, HD = 16, 4, 128
NREP = H // KV  # q heads per core
P = 128
NCORES = 8
CC_CHUNKS = C // P  # 16 contraction chunks
TT = 4  # t-blocks of 512
TB = T // TT  # 512
NB = 6  # j-blocks per core: q0..q3, k, v
SCALE = 1.0 / math.sqrt(HD)

f32 = mybir.dt.float32
f32r = mybir.dt.float32r

TRACE = False  # set True (with ntff shim installed) to get exec_time_ns

_cache = {}


def _build():
    if "nc" in _cache:
        return _cache["nc"]

    nc = bacc.Bacc("TRN2", target_bir_lowering=False, debug=False,
                   num_devices=NCORES)

    xT_d = nc.dram_tensor("xT", [P, CC_CHUNKS, T], f32r, kind="ExternalInput").ap()
    wT_d = nc.dram_tensor("wT", [P, CC_CHUNKS, NB * P], f32r, kind="ExternalInput").ap()
    cc_d = nc.dram_tensor("CC", [P, T], f32r, kind="ExternalInput").ap()
    ss_d = nc.dram_tensor("SS2", [P, T], f32r, kind="ExternalInput").ap()
    tri_d = nc.dram_tensor("tri", [P, P], f32r, kind="ExternalInput").ap()
    ones_d = nc.dram_tensor("ones", [P, P], f32r, kind="ExternalInput").ap()
    ident_d = nc.dram_tensor("ident", [P, P], f32r, kind="ExternalInput").ap()
    yT_d = nc.dram_tensor("yT", [NREP * P, T], f32, kind="ExternalOutput").ap()
    sums_d = nc.dram_tensor("sums", [NREP * TT, TB], f32, kind="ExternalOutput").ap()

    with tile.TileContext(nc) as tc:
        with (
            tc.tile_pool(name="wt", bufs=1) as wt_pool,
            tc.tile_pool(name="xt", bufs=4) as xt_pool,
            tc.tile_pool(name="qkvt", bufs=1) as qkv_pool,
            tc.tile_pool(name="freq", bufs=1) as freq_pool,
            tc.tile_pool(name="small", bufs=1) as small_pool,
            tc.tile_pool(name="vsb", bufs=1) as v_pool,
            tc.tile_pool(name="swp", bufs=3) as swp_pool,
            tc.tile_pool(name="ropetmp", bufs=3) as rt_pool,
            tc.tile_pool(name="expt", bufs=6) as exp_pool,
            tc.tile_pool(name="yout", bufs=2) as y_pool,
            tc.tile_pool(name="psum", bufs=8, space="PSUM") as psum_pool,
        ):
            # ---- resident tensors ----
            # all of tt0's x chunks first (11us), then weight quarters: the
            # PE sees one clean ~15us priming gap and is dense afterwards.
            xts_tt0 = []
            for cq in range(4):
                xt0 = xt_pool.tile([P, 4, TB], f32r, tag="xt", name=f"xt0_{cq}")
                nc.sync.dma_start(xt0[:], xT_d[:, cq * 4:(cq + 1) * 4, 0:TB])
                xts_tt0.append(xt0)
            wt_q = []
            for wq in range(4):
                wtq = wt_pool.tile([P, 4, NB * P], f32r, tag=f"wt{wq}", name=f"wt{wq}")
                nc.sync.dma_start(wtq[:], wT_d[:, wq * 4:(wq + 1) * 4, :])
                wt_q.append(wtq)

            # qkv^T blocks [128 d, 2048 t]: jb 0..3 = q heads (rope-permuted),
            # 4 = k (rope-permuted), 5 = v
            qkvT = [
                qkv_pool.tile([P, T], f32r, tag=f"qkv{jb}", name=f"qkv{jb}")
                for jb in range(NB)
            ]
            # V in s-major: [128 s, 16 s-chunk, 128 d]
            v_sb = v_pool.tile([P, CC_CHUNKS, P], f32r, tag="vsb")

            # late-loaded constants (needed only after proj tt0)
            ccs = freq_pool.tile([P, T], f32r, tag="cc")
            ss2 = freq_pool.tile([P, T], f32r, tag="ss")
            tri = small_pool.tile([P, P], f32r, tag="tri")
            ones = small_pool.tile([P, P], f32r, tag="ones")
            ident = small_pool.tile([P, P], f32r, tag="ident")
            _late = [(ccs, cc_d), (ss2, ss_d), (tri, tri_d), (ones, ones_d),
                     (ident, ident_d)]

            # ---- projection: qkv^T accumulated over 16 c-chunks ----
            for tt in range(TT):
                proj_psums = [
                    psum_pool.tile([P, TB], f32, tag="mm", name="proj_ps")
                    for _ in range(NB)
                ]
                for cq in range(4):  # c-chunk quarters, N=512 matmuls
                    if tt == 0:
                        xt = xts_tt0[cq]
                    else:
                        xt = xt_pool.tile([P, 4, TB], f32r, tag="xt", name="xt")
                        nc.sync.dma_start(
                            xt[:], xT_d[:, cq * 4:(cq + 1) * 4, tt * TB:(tt + 1) * TB]
                        )
                    for ci in range(4):
                        cc = cq * 4 + ci
                        for jb in range(NB):
                            nc.tensor.matmul(
                                proj_psums[jb][:],
                                wt_q[cq][:, ci, jb * P:(jb + 1) * P],
                                xt[:, ci, :],
                                start=(cc == 0),
                                stop=(cc == CC_CHUNKS - 1),
                            )
                if tt == 0:
                    for _tile, _src in _late:
                        nc.sync.dma_start(_tile[:], _src[:])
                    _late = []
                if True:
                    tsl = slice(tt * TB, (tt + 1) * TB)
                    for jb in range(NB):
                        if jb % 2 == 0:
                            nc.vector.tensor_copy(
                                qkvT[jb][:, tsl], proj_psums[jb][:]
                            )
                        else:
                            nc.scalar.copy(qkvT[jb][:, tsl], proj_psums[jb][:])
                    # V transpose for this chunk: v^T [d, s] -> v_sb [s, d]
                    for i in range(4):
                        sc = 4 * tt + i
                        trp = psum_pool.tile([P, TB], f32r, tag="mm", name="trp")
                        nc.tensor.transpose(
                            trp[:, :P], qkvT[5][:, sc * P:(sc + 1) * P], ident[:]
                        )
                        nc.vector.tensor_copy(v_sb[:, sc, :], trp[:, :P])
                    # RoPE for q0..q3 and k on this t-chunk
                    for jb in range(5):
                        swp = swp_pool.tile([P, TB], f32r, tag="swp", name="swp")
                        nc.sync.dma_start(swp[0:64, :], qkvT[jb][64:128, tsl])
                        nc.sync.dma_start(swp[64:128, :], qkvT[jb][0:64, tsl])
                        ta = rt_pool.tile([P, TB], f32r, tag="ta", name="ta")
                        tb_ = rt_pool.tile([P, TB], f32r, tag="tb", name="tb")
                        nc.vector.tensor_tensor(
                            ta[:], qkvT[jb][:, tsl], ccs[:, tsl], mybir.AluOpType.mult
                        )
                        nc.vector.tensor_tensor(
                            tb_[:], swp[:], ss2[:, tsl], mybir.AluOpType.mult
                        )
                        nc.vector.tensor_tensor(
                            qkvT[jb][:, tsl], ta[:], tb_[:], mybir.AluOpType.add
                        )

            # ---- attention, S^T orientation ----
            for tb in range(TT):
                for h in range(NREP):
                    psum_y = psum_pool.tile([P, TB], f32, tag="mm", name="psum_y")
                    psum_sum = psum_pool.tile([P, TB], f32, tag="mm", name="psum_sum")
                    nsc = 4 * (tb + 1)
                    for sc in range(nsc):
                        r = sc - 4 * tb  # >=0: diagonal-crossing chunk
                        col0 = r * P if r >= 0 else 0
                        psum_s = psum_pool.tile([P, TB], f32, tag="mm", name="psum_s")
                        nc.tensor.matmul(
                            psum_s[:, col0:],
                            qkvT[4][:, sc * P:(sc + 1) * P],
                            qkvT[h][:, tb * TB + col0:(tb + 1) * TB],
                            start=True,
                            stop=True,
                        )
                        expt = exp_pool.tile([P, TB], f32r, tag="expt", name="expt")
                        nc.scalar.activation(
                            expt[:, col0:],
                            psum_s[:, col0:],
                            mybir.ActivationFunctionType.Exp,
                            scale=SCALE,
                        )
                        if r >= 0:
                            nc.vector.tensor_tensor(
                                expt[:, col0:col0 + P],
                                expt[:, col0:col0 + P],
                                tri[:],
                                mybir.AluOpType.mult,
                            )
                        nc.tensor.matmul(
                            psum_sum[:, col0:],
                            ones[:],
                            expt[:, col0:],
                            start=(sc == 0),
                            stop=(sc == nsc - 1),
                        )
                        nc.tensor.matmul(
                            psum_y[:, col0:],
                            v_sb[:, sc, :],
                            expt[:, col0:],
                            start=(sc == 0),
                            stop=(sc == nsc - 1),
                        )
                    y_sb = y_pool.tile([P, TB], f32, tag="ysb", name="ysb")
                    nc.vector.tensor_copy(y_sb[:], psum_y[:])
                    nc.sync.dma_start(
                        yT_d[h * P:(h + 1) * P, tb * TB:(tb + 1) * TB], y_sb[:]
                    )
                    sums_sb = y_pool.tile([1, TB], f32, tag="sums", name="sums_sb")
                    nc.scalar.copy(sums_sb[:], psum_sum[0:1, :])
                    nc.sync.dma_start(
                        sums_d[h * TT + tb:h * TT + tb + 1, :], sums_sb[0:1, :]
                    )

    nc.compile()
    _cache["nc"] = nc
    return nc


def _host_prep(x, w_qkv, freqs_cos, freqs_sin):
    """Build per-core input maps (numpy, cheap)."""
    x = np.asarray(x, dtype=np.float32)
    w_qkv = np.asarray(w_qkv, dtype=np.float32)
    freqs_cos = np.asarray(freqs_cos, dtype=np.float32)
    freqs_sin = np.asarray(freqs_sin, dtype=np.float32)

    perm = np.concatenate([np.arange(0, HD, 2), np.arange(1, HD, 2)])

    xTs = []
    for b in range(B):
        xt = np.ascontiguousarray(
            x[b].T.reshape(CC_CHUNKS, P, T).transpose(1, 0, 2)
        )
        xTs.append(xt)

    cosT = freqs_cos.T  # [64, T]
    sinT = freqs_sin.T
    CCh = np.ascontiguousarray(np.concatenate([cosT, cosT], axis=0))
    SS2 = np.ascontiguousarray(np.concatenate([-sinT, sinT], axis=0))
    tri = np.triu(np.ones((P, P), dtype=np.float32))
    ones = np.ones((P, P), dtype=np.float32)
    ident = np.eye(P, dtype=np.float32)

    in_maps = []
    for core in range(NCORES):
        b, kv = divmod(core, KV)
        blocks = []
        for r in range(NREP):
            hrow = (kv * NREP + r) * HD
            blocks.append(w_qkv[hrow:hrow + HD][perm])
        blocks.append(w_qkv[H * HD + kv * HD:H * HD + (kv + 1) * HD][perm])
        blocks.append(
            w_qkv[(H + KV) * HD + kv * HD:(H + KV) * HD + (kv + 1) * HD]
        )
        w_shard = np.concatenate(blocks, axis=0)  # [768, C]
        wT = np.ascontiguousarray(
            w_shard.T.reshape(CC_CHUNKS, P, NB * P).transpose(1, 0, 2)
        )
        in_maps.append({
            "xT": xTs[b],
            "wT": wT,
            "CC": CCh,
            "SS2": SS2,
            "tri": tri,
            "ones": ones,
            "ident": ident,
        })
    return in_maps


def kernel(x, w_qkv, freqs_cos, freqs_sin):
    nc = _build()
    in_maps = _host_prep(x, w_qkv, freqs_cos, freqs_sin)
    res = run_bass_kernel_spmd(nc, in_maps, list(range(NCORES)), trace=TRACE)
    _cache["last_res"] = res

    y = np.empty((B, T, C), dtype=np.float32)
    for core in range(NCORES):
        b, kv = divmod(core, KV)
        yT = res.results[core]["yT"]  # [NREP*P, T] unnormalized
        sums = res.results[core]["sums"].reshape(NREP, T)  # per (h, t)
        yT = yT.reshape(NREP, P, T) / sums[:, None, :]
        y[b, :, kv * NREP * HD:(kv + 1) * NREP * HD] = (
            yT.reshape(NREP * P, T).T
        )
    return y


# revision 14
# speedup vs baseline: 1.0611x; 1.0312x over previous
"""Trainium2 Bass kernel for causal GQA self-attention (fused QKV + RoPE).

Problem: B=2, T=2048, C=2048, H=16 q-heads, KV=4 kv-heads, HD=128.
Sharding: 8 cores = (batch b, kv-group k). Each core computes the 4 q-heads
of one kv group for one batch element; outputs are disjoint slices of y.

Per-core device kernel (all fp32r matmuls, ~14-bit mantissa rounding):
  1. QKV projection qkv^T = W_shard @ x^T, d-major layout [j, t].
     Host pre-transposes x and W (and pre-permutes q/k head dims so RoPE
     becomes rotate-half instead of interleaved pairs).
  2. RoPE on q/k via SBUF->SBUF DMA partition swap + DVE mul/add.
  3. Attention in S^T orientation: scores^T[s,t] = K^T.T @ Q^T per
     (128 s-chunk x 512 t-block), exp on ScalarE, causal diagonal masked by a
     triangular multiply, row sums via an all-ones stationary matmul
     (partition reduction on the PE), PV with V stationary accumulating
     y^T[d,t] in PSUM, then normalize by reciprocal row sums.
Output per core: y^T [512, 2048]; host transposes and concatenates.
"""

import math

import numpy as np

import concourse.bass as bass
import concourse.mybir as mybir
import concourse.tile as tile
from concourse import bacc
from concourse.bass_utils import run_bass_kernel_spmd

B, T, C = 2, 2048, 2048
H, KV, HD = 16, 4, 128
NREP = H // KV  # q heads per core
P = 128
NCORES = 8
CC_CHUNKS = C // P  # 16 contraction chunks
TT = 4  # t-blocks of 512
TB = T // TT  # 512
NB = 6  # j-blocks per core: q0..q3, k, v
SCALE = 1.0 / math.sqrt(HD)

f32 = mybir.dt.float32
f32r = mybir.dt.float32r

TRACE = False  # set True (with ntff shim installed) to get exec_time_ns

_cache = {}


def _build():
    if "nc" in _cache:
        return _cache["nc"]

    nc = bacc.Bacc("TRN2", target_bir_lowering=False, debug=False,
                   num_devices=NCORES)

    xT_d = nc.dram_tensor("xT", [P, CC_CHUNKS, T], f32r, kind="ExternalInput").ap()
    wT_d = nc.dram_tensor("wT", [P, CC_CHUNKS, NB * P], f32r, kind="ExternalInput").ap()
    cc_d = nc.dram_tensor("CC", [P, T], f32r, kind="ExternalInput").ap()
    ss_d = nc.dram_tensor("SS2", [P, T], f32r, kind="ExternalInput").ap()
    tri_d = nc.dram_tensor("tri", [P, P], f32r, kind="ExternalInput").ap()
    ones_d = nc.dram_tensor("ones", [P, P], f32r, kind="ExternalInput").ap()
    ident_d = nc.dram_tensor("ident", [P, P], f32r, kind="ExternalInput").ap()
    yT_d = nc.dram_tensor("yT", [NREP * P, T], f32, kind="ExternalOutput").ap()
    sums_d = nc.dram_tensor("sums", [NREP * TT, TB], f32, kind="ExternalOutput").ap()

    with tile.TileContext(nc) as tc:
        with (
            tc.tile_pool(name="wt", bufs=1) as wt_pool,
            tc.tile_pool(name="xt", bufs=3) as xt_pool,
            tc.tile_pool(name="qkvt", bufs=1) as qkv_pool,
            tc.tile_pool(name="freq", bufs=1) as freq_pool,
            tc.tile_pool(name="small", bufs=1) as small_pool,
            tc.tile_pool(name="vsb", bufs=1) as v_pool,
            tc.tile_pool(name="swp", bufs=2) as swp_pool,
            tc.tile_pool(name="ropetmp", bufs=2) as rt_pool,
            tc.tile_pool(name="expt", bufs=6) as exp_pool,
            tc.tile_pool(name="yout", bufs=2) as y_pool,
            tc.tile_pool(name="psum", bufs=8, space="PSUM") as psum_pool,
        ):
            # ---- resident tensors ----
            wt_q = []
            for wq in range(4):
                wtq = wt_pool.tile([P, 4, NB * P], f32r, tag=f"wt{wq}", name=f"wt{wq}")
                nc.sync.dma_start(wtq[:], wT_d[:, wq * 4:(wq + 1) * 4, :])
                wt_q.append(wtq)

            # qkv^T blocks [128 d, 2048 t]: jb 0..3 = q heads (rope-permuted),
            # 4 = k (rope-permuted), 5 = v
            qkvT = [
                qkv_pool.tile([P, T], f32r, tag=f"qkv{jb}", name=f"qkv{jb}")
                for jb in range(NB)
            ]
            # V in s-major: [128 s, 16 s-chunk, 128 d]
            v_sb = v_pool.tile([P, CC_CHUNKS, P], f32r, tag="vsb")

            # late-loaded constants (needed only after proj tt0)
            ccs = freq_pool.tile([P, T], f32r, tag="cc")
            ss2 = freq_pool.tile([P, T], f32r, tag="ss")
            tri = small_pool.tile([P, P], f32r, tag="tri")
            ones = small_pool.tile([P, P], f32r, tag="ones")
            ident = small_pool.tile([P, P], f32r, tag="ident")
            _late = [(ccs, cc_d), (ss2, ss_d), (tri, tri_d), (ones, ones_d),
                     (ident, ident_d)]

            # ---- projection: qkv^T accumulated over 16 c-chunks ----
            for tt in range(TT):
                proj_psums = [
                    psum_pool.tile([P, TB], f32, tag="mm", name="proj_ps")
                    for _ in range(NB)
                ]
                for cq in range(4):  # c-chunk quarters, N=512 matmuls
                    xt = xt_pool.tile([P, 4, TB], f32r, tag="xt", name="xt")
                    nc.sync.dma_start(
                        xt[:], xT_d[:, cq * 4:(cq + 1) * 4, tt * TB:(tt + 1) * TB]
                    )
                    for ci in range(4):
                        cc = cq * 4 + ci
                        for jb in range(NB):
                            nc.tensor.matmul(
                                proj_psums[jb][:],
                                wt_q[cq][:, ci, jb * P:(jb + 1) * P],
                                xt[:, ci, :],
                                start=(cc == 0),
                                stop=(cc == CC_CHUNKS - 1),
                            )
                if tt == 0:
                    for _tile, _src in _late:
                        nc.sync.dma_start(_tile[:], _src[:])
                    _late = []
                if True:
                    tsl = slice(tt * TB, (tt + 1) * TB)
                    for jb in range(NB):
                        if jb % 2 == 0:
                            nc.vector.tensor_copy(
                                qkvT[jb][:, tsl], proj_psums[jb][:]
                            )
                        else:
                            nc.scalar.copy(qkvT[jb][:, tsl], proj_psums[jb][:])
                    # V transpose for this chunk: v^T [d, s] -> v_sb [s, d]
                    for i in range(4):
                        sc = 4 * tt + i
                        trp = psum_pool.tile([P, TB], f32r, tag="mm", name="trp")
                        nc.tensor.transpose(
                            trp[:, :P], qkvT[5][:, sc * P:(sc + 1) * P], ident[:]
                        )
                        nc.vector.tensor_copy(v_sb[:, sc, :], trp[:, :P])
                    # RoPE for q0..q3 and k on this t-chunk
                    for jb in range(5):
                        swp = swp_pool.tile([P, TB], f32r, tag="swp", name="swp")
                        nc.sync.dma_start(swp[0:64, :], qkvT[jb][64:128, tsl])
                        nc.sync.dma_start(swp[64:128, :], qkvT[jb][0:64, tsl])
                        ta = rt_pool.tile([P, TB], f32r, tag="ta", name="ta")
                        tb_ = rt_pool.tile([P, TB], f32r, tag="tb", name="tb")
                        nc.vector.tensor_tensor(
                            ta[:], qkvT[jb][:, tsl], ccs[:, tsl], mybir.AluOpType.mult
                        )
                        nc.vector.tensor_tensor(
                            tb_[:], swp[:], ss2[:, tsl], mybir.AluOpType.mult
                        )
                        nc.vector.tensor_tensor(
                            qkvT[jb][:, tsl], ta[:], tb_[:], mybir.AluOpType.add
                        )

            # ---- attention, S^T orientation ----
            for tb in range(TT):
                for h in range(NREP):
                    psum_y = psum_pool.tile([P, TB], f32, tag="mm", name="psum_y")
                    psum_sum = psum_pool.tile([P, TB], f32, tag="mm", name="psum_sum")
                    nsc = 4 * (tb + 1)
                    for sc in range(nsc):
                        r = sc - 4 * tb  # >=0: diagonal-crossing chunk
                        col0 = r * P if r >= 0 else 0
                        psum_s = psum_pool.tile([P, TB], f32, tag="mm", name="psum_s")
                        nc.tensor.matmul(
                            psum_s[:, col0:],
                            qkvT[4][:, sc * P:(sc + 1) * P],
                            qkvT[h][:, tb * TB + col0:(tb + 1) * TB],
                            start=True,
                            stop=True,
                        )
                        expt = exp_pool.tile([P, TB], f32r, tag="expt", name="expt")
                        nc.scalar.activation(
                            expt[:, col0:],
                            psum_s[:, col0:],
                            mybir.ActivationFunctionType.Exp,
                            scale=SCALE,
                        )
                        if r >= 0:
                            nc.vector.tensor_tensor(
                                expt[:, col0:col0 + P],
                                expt[:, col0:col0 + P],
                                tri[:],
                                mybir.AluOpType.mult,
                            )
                        nc.tensor.matmul(
                            psum_sum[:, col0:],
                            ones[:],
                            expt[:, col0:],
                            start=(sc == 0),
                            stop=(sc == nsc - 1),
                        )
                        nc.tensor.matmul(
                            psum_y[:, col0:],
                            v_sb[:, sc, :],
                            expt[:, col0:],
                            start=(sc == 0),
                            stop=(sc == nsc - 1),
                        )
                    y_sb = y_pool.tile([P, TB], f32, tag="ysb", name="ysb")
                    nc.vector.tensor_copy(y_sb[:], psum_y[:])
                    nc.sync.dma_start(
                        yT_d[h * P:(h + 1) * P, tb * TB:(tb + 1) * TB], y_sb[:]
                    )
                    sums_sb = y_pool.tile([1, TB], f32, tag="sums", name="sums_sb")
                    nc.scalar.copy(sums_sb[:], psum_sum[0:1, :])
                    nc.sync.dma_start(
                        sums_d[h * TT + tb:h * TT + tb + 1, :], sums_sb[0:1, :]
                    )

    nc.compile()
    _cache["nc"] = nc
    return nc


def _host_prep(x, w_qkv, freqs_cos, freqs_sin):
    """Build per-core input maps (numpy, cheap)."""
    x = np.asarray(x, dtype=np.float32)
    w_qkv = np.asarray(w_qkv, dtype=np.float32)
    freqs_cos = np.asarray(freqs_cos, dtype=np.float32)
    freqs_sin = np.asarray(freqs_sin, dtype=np.float32)

    perm = np.concatenate([np.arange(0, HD, 2), np.arange(1, HD, 2)])

    xTs = []
    for b in range(B):
        xt = np.ascontiguousarray(
            x[b].T.reshape(CC_CHUNKS, P, T).transpose(1, 0, 2)
        )
        xTs.append(xt)

    cosT = freqs_cos.T  # [64, T]
    sinT = freqs_sin.T
    CCh = np.ascontiguousarray(np.concatenate([cosT, cosT], axis=0))
    SS2 = np.ascontiguousarray(np.concatenate([-sinT, sinT], axis=0))
    tri = np.triu(np.ones((P, P), dtype=np.float32))
    ones = np.ones((P, P), dtype=np.float32)
    ident = np.eye(P, dtype=np.float32)

    in_maps = []
    for core in range(NCORES):
        b, kv = divmod(core, KV)
        blocks = []
        for r in range(NREP):
            hrow = (kv * NREP + r) * HD
            blocks.append(w_qkv[hrow:hrow + HD][perm])
        blocks.append(w_qkv[H * HD + kv * HD:H * HD + (kv + 1) * HD][perm])
        blocks.append(
            w_qkv[(H + KV) * HD + kv * HD:(H + KV) * HD + (kv + 1) * HD]
        )
        w_shard = np.concatenate(blocks, axis=0)  # [768, C]
        wT = np.ascontiguousarray(
            w_shard.T.reshape(CC_CHUNKS, P, NB * P).transpose(1, 0, 2)
        )
        in_maps.append({
            "xT": xTs[b],
            "wT": wT,
            "CC": CCh,
            "SS2": SS2,
            "tri": tri,
            "ones": ones,
            "ident": ident,
        })
    return in_maps


def kernel(x, w_qkv, freqs_cos, freqs_sin):
    nc = _build()
    in_maps = _host_prep(x, w_qkv, freqs_cos, freqs_sin)
    res = run_bass_kernel_spmd(nc, in_maps, list(range(NCORES)), trace=TRACE)
    _cache["last_res"] = res

    y = np.empty((B, T, C), dtype=np.float32)
    for core in range(NCORES):
        b, kv = divmod(core, KV)
        yT = res.results[core]["yT"]  # [NREP*P, T] unnormalized
        sums = res.results[core]["sums"].reshape(NREP, T)  # per (h, t)
        yT = yT.reshape(NREP, P, T) / sums[:, None, :]
        y[b, :, kv * NREP * HD:(kv + 1) * NREP * HD] = (
            yT.reshape(NREP * P, T).T
        )
    return y
